# revision 1
# baseline (speedup 1.0000x reference)
import numpy as np
import ml_dtypes
from contextlib import ExitStack

import concourse.bass as bass
import concourse.tile as tile
from concourse import bacc, mybir
from concourse.bass_utils import run_bass_kernel_spmd

F32 = mybir.dt.float32
BF16 = mybir.dt.bfloat16

B, S, DT = 128, 50, 300
AIN, VIN = 74, 35
AH, VH, TH = 32, 32, 128
TOUT = 128
PF = 768
NCORES = 8
BL = B // NCORES          # 16 batch per core (LSTM shard)
TPC = 17                  # t-values per core (8*17=136 >= 129, zero-padded)
SLAB = (AH + 1) * (VH + 1)  # 1089 = fusion rows per t
NCH = 9                   # ceil(1089/128) chunks per slab; last chunk 65 rows
LASTK = SLAB - 8 * 128    # 65
KLOC = TPC * SLAB         # 18513 k rows per core
GPERM = [0, 1, 3, 2]      # torch gate order i,f,g,o -> i,f,o,g

import os
BUILD_PHASE = int(os.environ.get("BUILD_PHASE", "8"))


def _phases():
    # partition phase of vT needed for each chunk c: v = (p + 128c) mod 33
    return [(128 * c) % 33 for c in range(NCH)]


def build_nc():
    nc = bacc.Bacc(None, target_bir_lowering=False)

    # ---- DRAM parameters (identical shapes on all cores) ----
    P = {}
    def par(name, shape, dt=F32):
        P[name] = nc.declare_dram_parameter(name, list(shape), dt, isOutput=False)
        return P[name]

    par("textxt", (DT, S * BL))          # [300, 800] (d, (s,b))
    par("wihT", (DT, 4 * TH))            # [300, 512] gate-permuted
    par("biaspp", (4, TH, 1))            # [4,128,1]
    par("whhT", (TH, 4 * TH))            # [128, 512] gate-permuted
    par("twTa", (TH, TH))                # t_w cols for global t 0..127
    par("twTb", (TH, NCORES))            # global t 128..135 (pads zero)
    par("tbla", (TH, 1))                 # bias col (1.0 at t==0)
    par("tblb", (NCORES, 1))
    par("audio", (B, AIN))
    par("video", (B, VIN))
    par("agam", (1, AIN)); par("abet", (1, AIN))
    par("vgam", (1, VIN)); par("vbet", (1, VIN))
    par("aw1T", (AIN, AH)); par("ab1", (AH, 1))
    par("aw2T", (AH, AH)); par("ab2", (AH, 1))
    par("aw3T", (AH, AH)); par("ab3", (AH, 1))
    par("vw1T", (VIN, VH)); par("vb1", (VH, 1))
    par("vw2T", (VH, VH)); par("vb2", (VH, 1))
    par("vw3T", (VH, VH)); par("vb3", (VH, 1))
    par("w1T", (KLOC, PF), BF16)         # [18513, 768] per-core pre-transposed slab
    par("selT", (AH + 1, NCH * 128))     # a-index selector: selT[a, c*128+p]=1 iff (128c+p)//33==a
    par("w2T", (PF, PF), BF16)
    par("b1", (1, PF)); par("b2", (1, PF))
    par("w3", (1, PF)); par("b3", (1, 1))
    par("id128", (128, 128))
    out_ext = nc.declare_dram_parameter("out", [B, 1], F32, isOutput=True)
    DBG = os.environ.get("KERNEL_DEBUG", "0") == "1"
    dbg = {}
    if DBG:
        for n, sh in [("dbg_h", (TH, BL)), ("dbg_th", (NCORES * TPC, BL)),
                      ("dbg_tT", (1, TPC * B)), ("dbg_av", (128, NCH * B)),
                      ("dbg_y1", (B, PF)), ("dbg_y1g", (B, PF)),
                      ("dbg_xp", (TH, 4 * S * BL))]:
            dbg[n] = nc.declare_dram_parameter(n, list(sh), F32, isOutput=True)

    with tile.TileContext(nc) as tc, ExitStack() as ctx:
        ep = ctx.enter_context
        const = ep(tc.tile_pool(name="const", bufs=1))
        w1p = ep(tc.tile_pool(name="w1p", bufs=56))
        work = ep(tc.tile_pool(name="work", bufs=2))
        dram = ep(tc.tile_pool(name="dram", bufs=1, space="DRAM"))

        def load(name, shape, dt=F32, pool=const):
            t = pool.tile(list(shape), dt, tag=name)
            nc.gpsimd.dma_start(t[:], P[name][:])
            return t

        ones1 = const.tile([1, 128], F32, tag="ones1")
        nc.vector.memset(ones1[:], 1.0)

        def pbcast(psum_pool, row_ap, n, tag="bc"):
            # replicate [1, n] row to [128, n] via K=1 outer product on PE
            bc = psum_pool.tile([128, n], F32, tag=tag)
            nc.tensor.matmul(bc[:], ones1[:], row_ap, start=True, stop=True)
            return bc

        # ---- small input loads ----
        idm = load("id128", (128, 128))
        txc, wihc = [], []
        kcs = [128, 128, DT - 256]
        for ci, kc in enumerate(kcs):
            t = const.tile([kc, S * BL], F32, tag=f"txc{ci}")
            nc.gpsimd.dma_start(t[:], P["textxt"][ci * 128:ci * 128 + kc, :])
            txc.append(t)
            w = const.tile([kc, 4 * TH], F32, tag=f"wihc{ci}")
            nc.gpsimd.dma_start(w[:], P["wihT"][ci * 128:ci * 128 + kc, :])
            wihc.append(w)
        whh = load("whhT", (TH, 4 * TH))
        biasj = []
        for j in range(4):
            bt = const.tile([TH, 1], F32, tag=f"biasj{j}")
            nc.gpsimd.dma_start(bt[:], P["biaspp"][j, :, :])
            biasj.append(bt)
        twTa = load("twTa", (TH, TH)); twTb = load("twTb", (TH, NCORES))
        tbla = load("tbla", (TH, 1)); tblb = load("tblb", (NCORES, 1))
        aud = load("audio", (B, AIN)); vid = load("video", (B, VIN))
        agam = load("agam", (1, AIN)); abet = load("abet", (1, AIN))
        vgam = load("vgam", (1, VIN)); vbet = load("vbet", (1, VIN))
        subw = {n: load(n, sh) for n, sh in [
            ("aw1T", (AIN, AH)), ("ab1", (AH, 1)), ("aw2T", (AH, AH)), ("ab2", (AH, 1)),
            ("aw3T", (AH, AH)), ("ab3", (AH, 1)),
            ("vw1T", (VIN, VH)), ("vb1", (VH, 1)), ("vw2T", (VH, VH)), ("vb2", (VH, 1)),
            ("vw3T", (VH, VH)), ("vb3", (VH, 1))]}
        w2c = []
        for j in range(6):
            t = const.tile([128, PF], BF16, tag=f"w2c{j}")
            nc.gpsimd.dma_start(t[:], P["w2T"][j * 128:(j + 1) * 128, :])
            w2c.append(t)
        b1r = load("b1", (1, PF)); b2r = load("b2", (1, PF))
        w3r = load("w3", (1, PF)); b3r = load("b3", (1, 1))

        # ---- W1 streaming DMAs (emitted early; pool slots throttle) ----
        w1tiles = []
        for tau in range(TPC):
            for c in range(NCH):
                kk = 128 if c < 8 else LASTK
                r0 = tau * SLAB + c * 128
                t = w1p.tile([128, PF], BF16, tag="w1chunk")
                nc.sync.dma_start(t[0:kk, :], P["w1T"][r0:r0 + kk, :])
                w1tiles.append(t)

        # ---- audio/video subnets (redundant, full batch), transposed outputs ----
        def subnet(x, gam, bet, nin, nh, w1, b1, w2, b2, w3, b3, out3):
            with tc.tile_pool(name="sn", bufs=2) as sn, \
                 tc.tile_pool(name="snp", bufs=2, space="PSUM") as snp:
                gb = pbcast(snp, gam[0:1, :], nin, tag="gbc")
                bb = pbcast(snp, bet[0:1, :], nin, tag="gbc")
                aff = sn.tile([B, nin], F32, tag="aff")
                nc.vector.tensor_mul(aff[:], x[:], gb[:])
                nc.vector.tensor_add(aff[:], aff[:], bb[:])
                pT = snp.tile([nin, B], F32, tag="pT")
                nc.tensor.transpose(pT[:], aff[:], idm[:])
                xT = sn.tile([nin, B], F32, tag="xT")
                nc.vector.tensor_copy(xT[:], pT[:])
                h = xT
                for wi, bi in ((w1, b1), (w2, b2), (w3, b3)):
                    hp = snp.tile([nh, B], F32, tag="hp")
                    nc.tensor.matmul(hp[:], wi[:], h[:], start=True, stop=True)
                    dst = out3 if wi is w3 else sn.tile([nh, B], F32, tag="hs")
                    nc.scalar.activation(dst[:], hp[:], mybir.ActivationFunctionType.Relu,
                                         bias=bi[:])
                    h = dst

        ah3 = const.tile([AH, B], F32, tag="ah3")
        subnet(aud, agam, abet, AIN, AH, subw["aw1T"], subw["ab1"], subw["aw2T"],
               subw["ab2"], subw["aw3T"], subw["ab3"], ah3)
        vh3 = const.tile([VH, B], F32, tag="vh3")
        subnet(vid, vgam, vbet, VIN, VH, subw["vw1T"], subw["vb1"], subw["vw2T"],
               subw["vb2"], subw["vw3T"], subw["vb3"], vh3)
        # vT/aT = [ones; h^T] (partition layout, via DMA partition shift)
        vT = const.tile([VH + 1, B], F32, tag="vT")
        nc.vector.memset(vT[0:1, :], 1.0)
        nc.gpsimd.dma_start(vT[1:VH + 1, :], vh3[:, :])
        aT33 = const.tile([AH + 1, B], F32, tag="aT33")
        nc.vector.memset(aT33[0:1, :], 1.0)
        nc.gpsimd.dma_start(aT33[1:AH + 1, :], ah3[:, :])
        selT = load("selT", (AH + 1, NCH * 128))

        # ---- x_proj: xpT_all[g%128, j, (s,b)] = (w_ih x + b)^T, gate-permuted ----
        xpT = const.tile([TH, 4, S * BL], F32, tag="xpT")
        with tc.tile_pool(name="xpp", bufs=2, space="PSUM") as xpp:
            for j in range(4):
                ps0 = xpp.tile([TH, 400], F32, tag="xps0")
                ps1 = xpp.tile([TH, 400], F32, tag="xps1")
                ps = [ps0, ps1]
                for ci in range(3):
                    for half in range(2):
                        nc.tensor.matmul(
                            ps[half][:],
                            wihc[ci][:, j * TH:(j + 1) * TH],
                            txc[ci][:, half * 400:(half + 1) * 400],
                            start=(ci == 0), stop=(ci == 2))
                for half in range(2):
                    nc.scalar.activation(xpT[:, j, half * 400:(half + 1) * 400],
                                         ps[half][:],
                                         mybir.ActivationFunctionType.Identity,
                                         bias=biasj[j][:])

        # ---- LSTM recurrence (transposed state, 16 batch cols) ----
        hT = const.tile([TH, BL], F32, tag="hT")
        cT = const.tile([TH, BL], F32, tag="cT")
        nc.vector.memset(hT[:], 0.0)
        nc.vector.memset(cT[:], 0.0)
        with tc.tile_pool(name="gp", bufs=2, space="PSUM") as gp, \
             tc.tile_pool(name="gs", bufs=2) as gs:
            for t in range(S):
                pg = gp.tile([TH, 4, BL], F32, tag="pg")
                for j in range(4):
                    nc.tensor.matmul(pg[:, j, :], whh[:, j * TH:(j + 1) * TH], hT[:],
                                     start=True, stop=True)
                ga = gs.tile([TH, 4, BL], F32, tag="ga")
                nc.vector.tensor_add(ga[:], pg[:], xpT[:, :, t * BL:(t + 1) * BL])
                nc.scalar.activation(ga[:, 0:3, :], ga[:, 0:3, :],
                                     mybir.ActivationFunctionType.Sigmoid)
                nc.scalar.activation(ga[:, 3, :], ga[:, 3, :],
                                     mybir.ActivationFunctionType.Tanh)
                t1 = gs.tile([TH, BL], F32, tag="t1")
                nc.vector.tensor_mul(t1[:], ga[:, 1, :], cT[:])      # sig(f)*c
                t2 = gs.tile([TH, BL], F32, tag="t2")
                nc.vector.tensor_mul(t2[:], ga[:, 0, :], ga[:, 3, :])  # sig(i)*tanh(g)
                nc.vector.tensor_add(cT[:], t1[:], t2[:])
                tc2 = gs.tile([TH, BL], F32, tag="tc2")
                nc.scalar.activation(tc2[:], cT[:], mybir.ActivationFunctionType.Tanh)
                nc.vector.tensor_mul(hT[:], ga[:, 2, :], tc2[:])     # sig(o)*tanh(c)

        # ---- all 136 text rows over local batch shard + AllToAll ----
        # After AllToAll, core i holds its own 17 t-rows over the FULL batch.
        tT = const.tile([1, TPC * B], F32, tag="tT")
        with tc.tile_pool(name="thp", bufs=2, space="PSUM") as thp:
            ph1 = thp.tile([TH, BL], F32, tag="ph1")
            nc.tensor.matmul(ph1[:], twTa[:], hT[:], start=True, stop=True)
            ths1 = work.tile([TH, BL], F32, tag="ths1")
            nc.scalar.activation(ths1[:], ph1[:], mybir.ActivationFunctionType.Identity,
                                 bias=tbla[:])
            ph2 = thp.tile([NCORES, BL], F32, tag="ph2")
            nc.tensor.matmul(ph2[:], twTb[:], hT[:], start=True, stop=True)
            ths2 = work.tile([NCORES, BL], F32, tag="ths2")
            nc.scalar.activation(ths2[:], ph2[:], mybir.ActivationFunctionType.Identity,
                                 bias=tblb[:])
        if DBG:
            nc.sync.dma_start(dbg["dbg_h"][:], hT[:])
            nc.sync.dma_start(dbg["dbg_xp"][:], xpT[:])
        agin = dram.tile([NCORES * TPC, BL], F32, tag="agin")
        agout = dram.tile([NCORES, TPC, BL], F32, tag="agout")
        nc.gpsimd.dma_start(agin[0:TH, :], ths1[:])
        nc.gpsimd.dma_start(agin[TH:TH + NCORES, :], ths2[:])
        nc.gpsimd.collective_compute(
            "AllToAll", mybir.AluOpType.bypass,
            replica_groups=[list(range(NCORES))],
            ins=[agin[:].opt()], outs=[agout[:].opt()])
        for tau in range(TPC):
            nc.gpsimd.dma_start(tT[0:1, tau * B:(tau + 1) * B], agout[:, tau, :])

        if DBG:
            nc.sync.dma_start(dbg["dbg_th"][0:TH, :], ths1[:])
            nc.sync.dma_start(dbg["dbg_th"][TH:TH + NCORES, :], ths2[:])
            nc.sync.dma_start(dbg["dbg_tT"][:], tT[:])
        # ---- avT [128, 9, 128]: av outer product, fusion-chunk layout ----
        # vT replicated/phase-shifted tiles (DMA partition shifts)
        phases = sorted(set(_phases()))
        vrep = {}
        for ph_ in phases:
            vr = const.tile([128, B], F32, tag=f"vrep{ph_}")
            p = 0
            v = ph_
            while p < 128:
                ln = min(33 - v, 128 - p)
                nc.gpsimd.dma_start(vr[p:p + ln, :], vT[v:v + ln, :])
                p += ln
                v = (v + ln) % 33
            vrep[ph_] = vr
        avT = const.tile([128, NCH, B], F32, tag="avT")
        with tc.tile_pool(name="bca", bufs=3, space="PSUM") as bca:
            for c in range(NCH):
                ph_ = (128 * c) % 33
                vr = vrep[ph_]
                arep = bca.tile([128, B], F32, tag="arep")
                nc.tensor.matmul(arep[:], selT[:, c * 128:(c + 1) * 128], aT33[:],
                                 start=True, stop=True)
                nc.vector.tensor_mul(avT[:, c, :], vr[:], arep[:])

        # ---- fusion slabs (bf16) + big matmul, accumulate y1 in PSUM ----
        y1sb = const.tile([B, PF], F32, tag="y1sb")
        with tc.tile_pool(name="y1pp", bufs=1, space="PSUM") as y1pp, \
             tc.tile_pool(name="fus", bufs=3) as fusp, \
             tc.tile_pool(name="tbc", bufs=3, space="PSUM") as tbcp:
            psY0 = y1pp.tile([B, 384], F32, tag="psY0")
            psY1 = y1pp.tile([B, 384], F32, tag="psY1")
            psYh = [psY0, psY1]
            wi = 0
            for tau in range(TPC):
                tb = tbcp.tile([128, B], F32, tag="tb")
                nc.tensor.matmul(tb[:], ones1[:], tT[0:1, tau * B:(tau + 1) * B],
                                 start=True, stop=True)
                fu = fusp.tile([128, NCH, B], BF16, tag="fu")
                for c in range(NCH):
                    nc.vector.tensor_mul(fu[:, c, :], avT[:, c, :], tb[:])
                for c in range(NCH):
                    kk = 128 if c < 8 else LASTK
                    w1t = w1tiles[wi]; wi += 1
                    first = (tau == 0 and c == 0)
                    last = (tau == TPC - 1 and c == NCH - 1)
                    nc.tensor.matmul(psYh[0][:], fu[0:kk, c, :], w1t[0:kk, 0:384],
                                     start=first, stop=last)
                    nc.tensor.matmul(psYh[1][:], fu[0:kk, c, :], w1t[0:kk, 384:768],
                                     start=first, stop=last)

            nc.vector.tensor_copy(y1sb[:, 0:384], psYh[0][:])
            nc.vector.tensor_copy(y1sb[:, 384:768], psYh[1][:])
        if DBG:
            nc.sync.dma_start(dbg["dbg_av"][:], avT[:])
            nc.sync.dma_start(dbg["dbg_y1"][:], y1sb[:])

        # ---- AllReduce y1 ----
        arin = dram.tile([B, PF], F32, tag="arin")
        arout = dram.tile([B, PF], F32, tag="arout")
        nc.gpsimd.dma_start(arin[:], y1sb[:])
        nc.gpsimd.collective_compute(
            "AllReduce", mybir.AluOpType.add,
            replica_groups=[list(range(NCORES))],
            ins=[arin[:].opt()], outs=[arout[:].opt()])
        y1g = const.tile([B, PF], F32, tag="y1g")
        nc.gpsimd.dma_start(y1g[:], arout[:])
        if DBG:
            nc.sync.dma_start(dbg["dbg_y1g"][:], y1g[:])

        # ---- epilogue: bias+relu, transpose, layer2, layer3, sigmoid ----
        y1r = const.tile([B, PF], F32, tag="y1r")
        with tc.tile_pool(name="ep1", bufs=2, space="PSUM") as ep1:
            for h in range(2):
                bh = ep1.tile([B, 384], F32, tag="epb")
                nc.tensor.matmul(bh[:], ones1[:], b1r[0:1, h * 384:(h + 1) * 384],
                                 start=True, stop=True)
                nc.vector.tensor_add(y1g[:, h * 384:(h + 1) * 384],
                                     y1g[:, h * 384:(h + 1) * 384], bh[:])
            nc.vector.tensor_scalar_max(y1r[:], y1g[:], 0.0)

        y1T = const.tile([128, 6, B], BF16, tag="y1T")
        with tc.tile_pool(name="trp", bufs=2, space="PSUM") as trp:
            for j in range(6):
                pt = trp.tile([128, B], F32, tag="pt")
                nc.tensor.transpose(pt[:], y1r[:, j * 128:(j + 1) * 128], idm[:])
                nc.vector.tensor_copy(y1T[:, j, :], pt[:])

        with tc.tile_pool(name="y2pp", bufs=1, space="PSUM") as y2pp:
            ps20 = y2pp.tile([B, 384], F32, tag="ps20")
            ps21 = y2pp.tile([B, 384], F32, tag="ps21")
            ps2h = [ps20, ps21]
            y2 = const.tile([B, PF], F32, tag="y2")
            for h in range(2):
                for j in range(6):
                    nc.tensor.matmul(ps2h[h][:], y1T[:, j, :],
                                     w2c[j][:, h * 384:(h + 1) * 384],
                                     start=(j == 0), stop=False)
                # bias via accumulating ones x b2 outer product, then relu from PSUM
                nc.tensor.matmul(ps2h[h][:], ones1[:], b2r[0:1, h * 384:(h + 1) * 384],
                                 start=False, stop=True)
                nc.vector.tensor_scalar_max(y2[:, h * 384:(h + 1) * 384],
                                            ps2h[h][:], 0.0)

        zb = const.tile([B, 1], F32, tag="zb")
        with tc.tile_pool(name="ep3", bufs=2, space="PSUM") as ep3:
            prod = const.tile([B, PF], F32, tag="prod")
            for h in range(2):
                wh = ep3.tile([B, 384], F32, tag="epb")
                nc.tensor.matmul(wh[:], ones1[:], w3r[0:1, h * 384:(h + 1) * 384],
                                 start=True, stop=True)
                nc.vector.tensor_mul(prod[:, h * 384:(h + 1) * 384],
                                     y2[:, h * 384:(h + 1) * 384], wh[:])
            scr = const.tile([B, PF], F32, tag="scr")
            zacc = const.tile([B, 1], F32, tag="zacc")
            nc.scalar.activation(scr[:], prod[:], mybir.ActivationFunctionType.Identity,
                                 accum_out=zacc[:])
            b3b = ep3.tile([B, 1], F32, tag="b3s")
            nc.tensor.matmul(b3b[:], ones1[:], b3r[0:1, :], start=True, stop=True)
            nc.vector.tensor_add(zb[:], zacc[:], b3b[:])
        zs = const.tile([B, 1], F32, tag="zs")
        nc.scalar.activation(zs[:], zb[:], mybir.ActivationFunctionType.Sigmoid)

        zf = const.tile([B, 1], F32, tag="zf")
        nc.vector.tensor_scalar(zf[:], zs[:], 6.0, -3.0,
                                mybir.AluOpType.mult, mybir.AluOpType.add)
        nc.sync.dma_start(out_ext[:], zf[:])

    nc.compile()
    return nc


def make_in_maps(inputs):
    f32 = lambda a: np.ascontiguousarray(a, dtype=np.float32)
    bf16 = lambda a: np.ascontiguousarray(a.astype(ml_dtypes.bfloat16))
    perm = np.concatenate([np.arange(g * TH, (g + 1) * TH) for g in GPERM])

    text_x = f32(inputs["text_x"])
    w_ihp = f32(inputs["w_ih"])[perm]          # [512, 300]
    w_hhp = f32(inputs["w_hh"])[perm]          # [512, 128]
    biaspp = (f32(inputs["b_ih"]) + f32(inputs["b_hh"]))[perm].reshape(4, TH, 1)
    t_w = f32(inputs["t_w"]); t_b = f32(inputs["t_b"])
    pf_w1 = np.asarray(inputs["pf_w1"], dtype=np.float32)

    common = dict(
        wihT=f32(w_ihp.T), biaspp=f32(biaspp), whhT=f32(w_hhp.T),
        audio=f32(inputs["audio_x"][:, 0, :]), video=f32(inputs["video_x"][:, 0, :]),
        agam=f32(inputs["a_gamma"]).reshape(1, -1), abet=f32(inputs["a_beta"]).reshape(1, -1),
        vgam=f32(inputs["v_gamma"]).reshape(1, -1), vbet=f32(inputs["v_beta"]).reshape(1, -1),
        aw1T=f32(inputs["a_w1"].T), ab1=f32(inputs["a_b1"]).reshape(-1, 1),
        aw2T=f32(inputs["a_w2"].T), ab2=f32(inputs["a_b2"]).reshape(-1, 1),
        aw3T=f32(inputs["a_w3"].T), ab3=f32(inputs["a_b3"]).reshape(-1, 1),
        vw1T=f32(inputs["v_w1"].T), vb1=f32(inputs["v_b1"]).reshape(-1, 1),
        vw2T=f32(inputs["v_w2"].T), vb2=f32(inputs["v_b2"]).reshape(-1, 1),
        vw3T=f32(inputs["v_w3"].T), vb3=f32(inputs["v_b3"]).reshape(-1, 1),
        w2T=bf16(f32(inputs["pf_w2"]).T),
        b1=f32(inputs["pf_b1"]).reshape(1, -1), b2=f32(inputs["pf_b2"]).reshape(1, -1),
        w3=f32(inputs["pf_w3"]).reshape(1, -1), b3=f32(inputs["pf_b3"]).reshape(1, 1),
        id128=np.eye(128, dtype=np.float32),
    )
    sel = np.zeros((AH + 1, NCH * 128), np.float32)
    for r in range(SLAB):
        sel[r // (VH + 1), r] = 1.0
    common["selT"] = sel
    twTall = np.zeros((TH, NCORES * TPC), np.float32)
    tblall = np.zeros((NCORES * TPC, 1), np.float32)
    tblall[0, 0] = 1.0
    twTall[:, 1:TOUT + 1] = t_w.T
    tblall[1:TOUT + 1, 0] = t_b
    common["twTa"] = f32(twTall[:, 0:TH])
    common["twTb"] = f32(twTall[:, TH:TH + NCORES])
    common["tbla"] = f32(tblall[0:TH])
    common["tblb"] = f32(tblall[TH:TH + NCORES])

    in_maps = []
    for i in range(NCORES):
        m = dict(common)
        sh = text_x[i * BL:(i + 1) * BL]                      # [16, 50, 300]
        m["textxt"] = f32(sh.transpose(2, 1, 0).reshape(DT, S * BL))
        # per-core W1 slab: global t in [i*TPC, (i+1)*TPC), zero-padded past t=128
        w1t = np.zeros((KLOC, PF), np.float32)
        for tau in range(TPC):
            tg = i * TPC + tau
            if tg < (TOUT + 1):
                w1t[tau * SLAB:(tau + 1) * SLAB] = pf_w1[:, tg * SLAB:(tg + 1) * SLAB].T
        m["w1T"] = bf16(w1t)
        in_maps.append(m)
    return in_maps


_CACHE = {}


def kernel(**inputs):
    if "nc" not in _CACHE:
        _CACHE["nc"] = build_nc()
    nc = _CACHE["nc"]
    in_maps = make_in_maps(inputs)
    res = run_bass_kernel_spmd(nc, in_maps, core_ids=list(range(NCORES)))
    return np.asarray(res.results[0]["out"], dtype=np.float32)


if __name__ == "__main__":
    import reference
    inputs = {k: np.asarray(v) for k, v in reference.setup_inputs().items()}
    out = kernel(**inputs)
    exp = np.asarray(reference.reference(**inputs))
    err = np.linalg.norm(out - exp) / np.linalg.norm(exp)
    print("Relative error:", err)



# revision 14
# speedup vs baseline: 1.4775x; 1.4775x over previous
import numpy as np
import ml_dtypes
from contextlib import ExitStack

import concourse.bass as bass
import concourse.tile as tile
from concourse import bacc, mybir
from concourse.bass_utils import run_bass_kernel_spmd

F32 = mybir.dt.float32
BF16 = mybir.dt.bfloat16

B, S, DT = 128, 50, 300
AIN, VIN = 74, 35
AH, VH, TH = 32, 32, 128
TOUT = 128
PF = 768
NCORES = 8
BL = B // NCORES          # 16 batch per core (LSTM shard)
TPC = 17                  # t-values per core (8*17=136 >= 129, zero-padded)
SLAB = (AH + 1) * (VH + 1)  # 1089 = fusion rows per t
NCH = 9                   # ceil(1089/128) chunks per slab; last chunk 65 rows
LASTK = SLAB - 8 * 128    # 65
KLOC = TPC * SLAB         # 18513 k rows per core
GPERM = [0, 1, 3, 2]      # torch gate order i,f,g,o -> i,f,o,g

import os
BUILD_PHASE = int(os.environ.get("BUILD_PHASE", "8"))


def _phases():
    # partition phase of vT needed for each chunk c: v = (p + 128c) mod 33
    return [(128 * c) % 33 for c in range(NCH)]


def build_nc():
    nc = bacc.Bacc(None, target_bir_lowering=False)

    # ---- DRAM parameters (identical shapes on all cores) ----
    P = {}
    def par(name, shape, dt=F32):
        P[name] = nc.declare_dram_parameter(name, list(shape), dt, isOutput=False)
        return P[name]

    par("textxt", (DT, S * BL), BF16)    # [300, 800] (d, (s,b))
    par("wihT", (DT, 4 * TH), BF16)      # [300, 512] gate-permuted
    par("biaspp", (4, TH, 1))            # [4,128,1]
    par("whhT", (TH, 4 * TH), BF16)      # [128, 512] gate-permuted
    par("twTa", (TH, TH), BF16)          # t_w cols for global t 0..127
    par("twTb", (TH, NCORES), BF16)      # global t 128..135 (pads zero)
    par("tbla", (TH, 1))                 # bias col (1.0 at t==0)
    par("tblb", (NCORES, 1))
    par("audio", (B, AIN))
    par("video", (B, VIN))
    par("agam", (1, AIN)); par("abet", (1, AIN))
    par("vgam", (1, VIN)); par("vbet", (1, VIN))
    par("aw1T", (AIN, AH)); par("ab1", (AH, 1))
    par("aw2T", (AH, AH)); par("ab2", (AH, 1))
    par("aw3T", (AH, AH)); par("ab3", (AH, 1))
    par("vw1T", (VIN, VH)); par("vb1", (VH, 1))
    par("vw2T", (VH, VH)); par("vb2", (VH, 1))
    par("vw3T", (VH, VH)); par("vb3", (VH, 1))
    par("w1T", (KLOC, PF), BF16)         # [18513, 768] per-core pre-transposed slab
    par("selT", (AH + 1, NCH * 128))     # a-index selector: selT[a, c*128+p]=1 iff (128c+p)//33==a
    par("w2T", (PF, PF), BF16)
    par("b1", (1, PF)); par("b2", (1, PF))
    par("w3", (1, PF)); par("b3", (1, 1))
    par("id128", (128, 128))
    out_ext = nc.declare_dram_parameter("out", [B, 1], F32, isOutput=True)
    DBG = os.environ.get("KERNEL_DEBUG", "0") == "1"
    dbg = {}
    if DBG:
        for n, sh in [("dbg_h", (TH, BL)), ("dbg_th", (NCORES * TPC, BL)),
                      ("dbg_tT", (1, TPC * B)), ("dbg_av", (128, NCH * B)),
                      ("dbg_y1", (B, PF)), ("dbg_y1g", (B, PF)),
                      ("dbg_xp", (TH, 4 * S * BL))]:
            dbg[n] = nc.declare_dram_parameter(n, list(sh), F32, isOutput=True)

    with tile.TileContext(nc) as tc, ExitStack() as ctx:
        ep = ctx.enter_context
        const = ep(tc.tile_pool(name="const", bufs=1))
        w1p = ep(tc.tile_pool(name="w1p", bufs=56))
        work = ep(tc.tile_pool(name="work", bufs=2))
        dram = ep(tc.tile_pool(name="dram", bufs=1, space="DRAM"))

        def load(name, shape, dt=F32, pool=const):
            t = pool.tile(list(shape), dt, tag=name)
            nc.gpsimd.dma_start(t[:], P[name][:])
            return t

        ones1 = const.tile([1, 128], F32, tag="ones1")
        nc.vector.memset(ones1[:], 1.0)
        ones1b = const.tile([1, 128], BF16, tag="ones1b")
        nc.vector.memset(ones1b[:], 1.0)

        def pbcast(psum_pool, row_ap, n, tag="bc"):
            # replicate [1, n] row to [128, n] via K=1 outer product on PE
            bc = psum_pool.tile([128, n], F32, tag=tag)
            nc.tensor.matmul(bc[:], ones1[:], row_ap, start=True, stop=True)
            return bc

        # ---- small input loads ----
        idm = load("id128", (128, 128))
        txc, wihc = [], []
        kcs = [128, 128, DT - 256]
        for ci, kc in enumerate(kcs):
            t = const.tile([kc, S * BL], BF16, tag=f"txc{ci}")
            nc.gpsimd.dma_start(t[:], P["textxt"][ci * 128:ci * 128 + kc, :])
            txc.append(t)
            w = const.tile([kc, 4 * TH], BF16, tag=f"wihc{ci}")
            nc.gpsimd.dma_start(w[:], P["wihT"][ci * 128:ci * 128 + kc, :])
            wihc.append(w)
        whh = load("whhT", (TH, 4 * TH), BF16)
        biasj = []
        for j in range(4):
            bt = const.tile([TH, 1], F32, tag=f"biasj{j}")
            nc.gpsimd.dma_start(bt[:], P["biaspp"][j, :, :])
            biasj.append(bt)
        twTa = load("twTa", (TH, TH), BF16); twTb = load("twTb", (TH, NCORES), BF16)
        tbla = load("tbla", (TH, 1)); tblb = load("tblb", (NCORES, 1))
        aud = load("audio", (B, AIN)); vid = load("video", (B, VIN))
        agam = load("agam", (1, AIN)); abet = load("abet", (1, AIN))
        vgam = load("vgam", (1, VIN)); vbet = load("vbet", (1, VIN))
        subw = {n: load(n, sh) for n, sh in [
            ("aw1T", (AIN, AH)), ("ab1", (AH, 1)), ("aw2T", (AH, AH)), ("ab2", (AH, 1)),
            ("aw3T", (AH, AH)), ("ab3", (AH, 1)),
            ("vw1T", (VIN, VH)), ("vb1", (VH, 1)), ("vw2T", (VH, VH)), ("vb2", (VH, 1)),
            ("vw3T", (VH, VH)), ("vb3", (VH, 1))]}
        w2c = []
        for j in range(6):
            t = const.tile([128, PF], BF16, tag=f"w2c{j}")
            nc.gpsimd.dma_start(t[:], P["w2T"][j * 128:(j + 1) * 128, :])
            w2c.append(t)
        b1r = load("b1", (1, PF)); b2r = load("b2", (1, PF))
        w3r = load("w3", (1, PF)); b3r = load("b3", (1, 1))

        # ---- W1 streaming DMAs (emitted early; pool slots throttle) ----
        w1tiles = []
        for tau in range(TPC):
            for c in range(NCH):
                kk = 128 if c < 8 else LASTK
                r0 = tau * SLAB + c * 128
                t = w1p.tile([128, PF], BF16, tag="w1chunk")
                nc.sync.dma_start(t[0:kk, :], P["w1T"][r0:r0 + kk, :])
                w1tiles.append(t)

        # ---- audio/video subnets (redundant, full batch), transposed outputs ----
        def subnet(x, gam, bet, nin, nh, w1, b1, w2, b2, w3, b3, out3):
            with tc.tile_pool(name="sn", bufs=2) as sn, \
                 tc.tile_pool(name="snp", bufs=2, space="PSUM") as snp:
                gb = pbcast(snp, gam[0:1, :], nin, tag="gbc")
                bb = pbcast(snp, bet[0:1, :], nin, tag="gbc")
                aff = sn.tile([B, nin], F32, tag="aff")
                nc.vector.tensor_mul(aff[:], x[:], gb[:])
                nc.vector.tensor_add(aff[:], aff[:], bb[:])
                pT = snp.tile([nin, B], F32, tag="pT")
                nc.tensor.transpose(pT[:], aff[:], idm[:])
                xT = sn.tile([nin, B], F32, tag="xT")
                nc.vector.tensor_copy(xT[:], pT[:])
                h = xT
                for wi, bi in ((w1, b1), (w2, b2), (w3, b3)):
                    hp = snp.tile([nh, B], F32, tag="hp")
                    nc.tensor.matmul(hp[:], wi[:], h[:], start=True, stop=True)
                    dst = out3 if wi is w3 else sn.tile([nh, B], F32, tag="hs")
                    nc.scalar.activation(dst[:], hp[:], mybir.ActivationFunctionType.Relu,
                                         bias=bi[:])
                    h = dst

        ah3 = const.tile([AH, B], F32, tag="ah3")
        subnet(aud, agam, abet, AIN, AH, subw["aw1T"], subw["ab1"], subw["aw2T"],
               subw["ab2"], subw["aw3T"], subw["ab3"], ah3)
        vh3 = const.tile([VH, B], F32, tag="vh3")
        subnet(vid, vgam, vbet, VIN, VH, subw["vw1T"], subw["vb1"], subw["vw2T"],
               subw["vb2"], subw["vw3T"], subw["vb3"], vh3)
        # vT/aT = [ones; h^T] (partition layout, via DMA partition shift)
        vT = const.tile([VH + 1, B], F32, tag="vT")
        nc.vector.memset(vT[0:1, :], 1.0)
        nc.gpsimd.dma_start(vT[1:VH + 1, :], vh3[:, :])
        aT33 = const.tile([AH + 1, B], F32, tag="aT33")
        nc.vector.memset(aT33[0:1, :], 1.0)
        nc.gpsimd.dma_start(aT33[1:AH + 1, :], ah3[:, :])
        selT = load("selT", (AH + 1, NCH * 128))

        # ---- x_proj: xpT_all[g%128, j, (s,b)] = (w_ih x + b)^T, gate-permuted ----
        xpT = const.tile([TH, 4, S * BL], F32, tag="xpT")
        with tc.tile_pool(name="xpp", bufs=2, space="PSUM") as xpp:
            for j in range(4):
                ps0 = xpp.tile([TH, 400], F32, tag="xps0")
                ps1 = xpp.tile([TH, 400], F32, tag="xps1")
                ps = [ps0, ps1]
                for ci in range(3):
                    for half in range(2):
                        nc.tensor.matmul(
                            ps[half][:],
                            wihc[ci][:, j * TH:(j + 1) * TH],
                            txc[ci][:, half * 400:(half + 1) * 400],
                            start=(ci == 0), stop=(ci == 2))
                for half in range(2):
                    nc.scalar.activation(xpT[:, j, half * 400:(half + 1) * 400],
                                         ps[half][:],
                                         mybir.ActivationFunctionType.Identity,
                                         bias=biasj[j][:])

        # ---- LSTM recurrence (transposed state, 16 batch cols) ----
        hT = const.tile([TH, BL], BF16, tag="hT")
        cT = const.tile([TH, BL], F32, tag="cT")
        nc.vector.memset(hT[:], 0.0)
        nc.vector.memset(cT[:], 0.0)
        with tc.tile_pool(name="gp", bufs=2, space="PSUM") as gp, \
             tc.tile_pool(name="gs", bufs=2) as gs:
            for t in range(S):
                pg = gp.tile([TH, 4, BL], F32, tag="pg")
                for j in range(4):
                    nc.tensor.matmul(pg[:, j, :], whh[:, j * TH:(j + 1) * TH], hT[:],
                                     start=True, stop=True)
                ga = gs.tile([TH, 4, BL], F32, tag="ga")
                nc.vector.tensor_add(ga[:], pg[:], xpT[:, :, t * BL:(t + 1) * BL])
                nc.scalar.activation(ga[:, 0:3, :], ga[:, 0:3, :],
                                     mybir.ActivationFunctionType.Sigmoid)
                nc.scalar.activation(ga[:, 3, :], ga[:, 3, :],
                                     mybir.ActivationFunctionType.Tanh)
                t1 = gs.tile([TH, BL], F32, tag="t1")
                nc.vector.tensor_mul(t1[:], ga[:, 1, :], cT[:])      # sig(f)*c
                t2 = gs.tile([TH, BL], F32, tag="t2")
                nc.vector.tensor_mul(t2[:], ga[:, 0, :], ga[:, 3, :])  # sig(i)*tanh(g)
                nc.vector.tensor_add(cT[:], t1[:], t2[:])
                tc2 = gs.tile([TH, BL], F32, tag="tc2")
                nc.scalar.activation(tc2[:], cT[:], mybir.ActivationFunctionType.Tanh)
                nc.vector.tensor_mul(hT[:], ga[:, 2, :], tc2[:])     # sig(o)*tanh(c)

        # ---- all 136 text rows over local batch shard + AllToAll ----
        # After AllToAll, core i holds its own 17 t-rows over the FULL batch.
        tT = const.tile([1, TPC * B], BF16, tag="tT")
        with tc.tile_pool(name="thp", bufs=2, space="PSUM") as thp:
            ph1 = thp.tile([TH, BL], F32, tag="ph1")
            nc.tensor.matmul(ph1[:], twTa[:], hT[:], start=True, stop=True)
            ths1 = work.tile([TH, BL], BF16, tag="ths1")
            nc.scalar.activation(ths1[:], ph1[:], mybir.ActivationFunctionType.Identity,
                                 bias=tbla[:])
            ph2 = thp.tile([NCORES, BL], F32, tag="ph2")
            nc.tensor.matmul(ph2[:], twTb[:], hT[:], start=True, stop=True)
            ths2 = work.tile([NCORES, BL], BF16, tag="ths2")
            nc.scalar.activation(ths2[:], ph2[:], mybir.ActivationFunctionType.Identity,
                                 bias=tblb[:])
        if DBG:
            nc.sync.dma_start(dbg["dbg_h"][:], hT[:])
            nc.sync.dma_start(dbg["dbg_xp"][:], xpT[:])
        agin = dram.tile([NCORES * TPC, BL], BF16, tag="agin")
        agout = dram.tile([NCORES, TPC, BL], BF16, tag="agout")
        nc.gpsimd.dma_start(agin[0:TH, :], ths1[:])
        nc.gpsimd.dma_start(agin[TH:TH + NCORES, :], ths2[:])
        nc.gpsimd.collective_compute(
            "AllToAll", mybir.AluOpType.bypass,
            replica_groups=[list(range(NCORES))],
            ins=[agin[:].opt()], outs=[agout[:].opt()])
        for tau in range(TPC):
            nc.gpsimd.dma_start(tT[0:1, tau * B:(tau + 1) * B], agout[:, tau, :])

        if DBG:
            nc.sync.dma_start(dbg["dbg_th"][0:TH, :], ths1[:])
            nc.sync.dma_start(dbg["dbg_th"][TH:TH + NCORES, :], ths2[:])
            nc.sync.dma_start(dbg["dbg_tT"][:], tT[:])
        # ---- avT [128, 9, 128]: av outer product, fusion-chunk layout ----
        # vT replicated/phase-shifted tiles (DMA partition shifts)
        phases = sorted(set(_phases()))
        vrep = {}
        for ph_ in phases:
            vr = const.tile([128, B], F32, tag=f"vrep{ph_}")
            p = 0
            v = ph_
            while p < 128:
                ln = min(33 - v, 128 - p)
                nc.gpsimd.dma_start(vr[p:p + ln, :], vT[v:v + ln, :])
                p += ln
                v = (v + ln) % 33
            vrep[ph_] = vr
        avT = const.tile([128, NCH, B], F32, tag="avT")
        with tc.tile_pool(name="bca", bufs=3, space="PSUM") as bca:
            for c in range(NCH):
                ph_ = (128 * c) % 33
                vr = vrep[ph_]
                arep = bca.tile([128, B], F32, tag="arep")
                nc.tensor.matmul(arep[:], selT[:, c * 128:(c + 1) * 128], aT33[:],
                                 start=True, stop=True)
                nc.vector.tensor_mul(avT[:, c, :], vr[:], arep[:])

        # ---- fusion slabs (bf16) + big matmul, accumulate y1 in PSUM ----
        y1sb = const.tile([B, PF], F32, tag="y1sb")
        with tc.tile_pool(name="y1pp", bufs=1, space="PSUM") as y1pp, \
             tc.tile_pool(name="fus", bufs=3) as fusp, \
             tc.tile_pool(name="tbc", bufs=3, space="PSUM") as tbcp:
            psY0 = y1pp.tile([B, 384], F32, tag="psY0")
            psY1 = y1pp.tile([B, 384], F32, tag="psY1")
            psYh = [psY0, psY1]
            wi = 0
            for tau in range(TPC):
                tb = tbcp.tile([128, B], F32, tag="tb")
                nc.tensor.matmul(tb[:], ones1b[:], tT[0:1, tau * B:(tau + 1) * B],
                                 start=True, stop=True)
                fu = fusp.tile([128, NCH, B], BF16, tag="fu")
                for c in range(NCH):
                    nc.vector.tensor_mul(fu[:, c, :], avT[:, c, :], tb[:])
                for c in range(NCH):
                    kk = 128 if c < 8 else LASTK
                    w1t = w1tiles[wi]; wi += 1
                    first = (tau == 0 and c == 0)
                    last = (tau == TPC - 1 and c == NCH - 1)
                    nc.tensor.matmul(psYh[0][:], fu[0:kk, c, :], w1t[0:kk, 0:384],
                                     start=first, stop=last)
                    nc.tensor.matmul(psYh[1][:], fu[0:kk, c, :], w1t[0:kk, 384:768],
                                     start=first, stop=last)

            nc.vector.tensor_copy(y1sb[:, 0:384], psYh[0][:])
            nc.vector.tensor_copy(y1sb[:, 384:768], psYh[1][:])
        if DBG:
            nc.sync.dma_start(dbg["dbg_av"][:], avT[:])
            nc.sync.dma_start(dbg["dbg_y1"][:], y1sb[:])

        # ---- AllReduce y1 ----
        arin = dram.tile([B, PF], F32, tag="arin")
        arout = dram.tile([B, PF], F32, tag="arout")
        nc.gpsimd.dma_start(arin[:], y1sb[:])
        nc.gpsimd.collective_compute(
            "AllReduce", mybir.AluOpType.add,
            replica_groups=[list(range(NCORES))],
            ins=[arin[:].opt()], outs=[arout[:].opt()])
        y1g = const.tile([B, PF], F32, tag="y1g")
        nc.gpsimd.dma_start(y1g[:], arout[:])
        if DBG:
            nc.sync.dma_start(dbg["dbg_y1g"][:], y1g[:])

        # ---- epilogue: bias+relu, transpose, layer2, layer3, sigmoid ----
        y1r = const.tile([B, PF], F32, tag="y1r")
        with tc.tile_pool(name="ep1", bufs=2, space="PSUM") as ep1:
            for h in range(2):
                bh = ep1.tile([B, 384], F32, tag="epb")
                nc.tensor.matmul(bh[:], ones1[:], b1r[0:1, h * 384:(h + 1) * 384],
                                 start=True, stop=True)
                nc.vector.tensor_add(y1g[:, h * 384:(h + 1) * 384],
                                     y1g[:, h * 384:(h + 1) * 384], bh[:])
            nc.vector.tensor_scalar_max(y1r[:], y1g[:], 0.0)

        y1T = const.tile([128, 6, B], BF16, tag="y1T")
        with tc.tile_pool(name="trp", bufs=2, space="PSUM") as trp:
            for j in range(6):
                pt = trp.tile([128, B], F32, tag="pt")
                nc.tensor.transpose(pt[:], y1r[:, j * 128:(j + 1) * 128], idm[:])
                nc.vector.tensor_copy(y1T[:, j, :], pt[:])

        with tc.tile_pool(name="y2pp", bufs=1, space="PSUM") as y2pp:
            ps20 = y2pp.tile([B, 384], F32, tag="ps20")
            ps21 = y2pp.tile([B, 384], F32, tag="ps21")
            ps2h = [ps20, ps21]
            y2 = const.tile([B, PF], F32, tag="y2")
            for h in range(2):
                for j in range(6):
                    nc.tensor.matmul(ps2h[h][:], y1T[:, j, :],
                                     w2c[j][:, h * 384:(h + 1) * 384],
                                     start=(j == 0), stop=False)
                # bias via accumulating ones x b2 outer product, then relu from PSUM
                nc.tensor.matmul(ps2h[h][:], ones1[:], b2r[0:1, h * 384:(h + 1) * 384],
                                 start=False, stop=True)
                nc.vector.tensor_scalar_max(y2[:, h * 384:(h + 1) * 384],
                                            ps2h[h][:], 0.0)

        zb = const.tile([B, 1], F32, tag="zb")
        with tc.tile_pool(name="ep3", bufs=2, space="PSUM") as ep3:
            prod = const.tile([B, PF], F32, tag="prod")
            for h in range(2):
                wh = ep3.tile([B, 384], F32, tag="epb")
                nc.tensor.matmul(wh[:], ones1[:], w3r[0:1, h * 384:(h + 1) * 384],
                                 start=True, stop=True)
                nc.vector.tensor_mul(prod[:, h * 384:(h + 1) * 384],
                                     y2[:, h * 384:(h + 1) * 384], wh[:])
            scr = const.tile([B, PF], F32, tag="scr")
            zacc = const.tile([B, 1], F32, tag="zacc")
            nc.scalar.activation(scr[:], prod[:], mybir.ActivationFunctionType.Identity,
                                 accum_out=zacc[:])
            b3b = ep3.tile([B, 1], F32, tag="b3s")
            nc.tensor.matmul(b3b[:], ones1[:], b3r[0:1, :], start=True, stop=True)
            nc.vector.tensor_add(zb[:], zacc[:], b3b[:])
        zs = const.tile([B, 1], F32, tag="zs")
        nc.scalar.activation(zs[:], zb[:], mybir.ActivationFunctionType.Sigmoid)

        zf = const.tile([B, 1], F32, tag="zf")
        nc.vector.tensor_scalar(zf[:], zs[:], 6.0, -3.0,
                                mybir.AluOpType.mult, mybir.AluOpType.add)
        nc.sync.dma_start(out_ext[:], zf[:])

    nc.compile()
    return nc


def make_in_maps(inputs):
    f32 = lambda a: np.ascontiguousarray(a, dtype=np.float32)
    bf16 = lambda a: np.ascontiguousarray(a.astype(ml_dtypes.bfloat16))
    perm = np.concatenate([np.arange(g * TH, (g + 1) * TH) for g in GPERM])

    text_x = f32(inputs["text_x"])
    w_ihp = f32(inputs["w_ih"])[perm]          # [512, 300]
    w_hhp = f32(inputs["w_hh"])[perm]          # [512, 128]
    biaspp = (f32(inputs["b_ih"]) + f32(inputs["b_hh"]))[perm].reshape(4, TH, 1)
    t_w = f32(inputs["t_w"]); t_b = f32(inputs["t_b"])
    pf_w1 = np.asarray(inputs["pf_w1"], dtype=np.float32)

    common = dict(
        wihT=bf16(w_ihp.T), biaspp=f32(biaspp), whhT=bf16(w_hhp.T),
        audio=f32(inputs["audio_x"][:, 0, :]), video=f32(inputs["video_x"][:, 0, :]),
        agam=f32(inputs["a_gamma"]).reshape(1, -1), abet=f32(inputs["a_beta"]).reshape(1, -1),
        vgam=f32(inputs["v_gamma"]).reshape(1, -1), vbet=f32(inputs["v_beta"]).reshape(1, -1),
        aw1T=f32(inputs["a_w1"].T), ab1=f32(inputs["a_b1"]).reshape(-1, 1),
        aw2T=f32(inputs["a_w2"].T), ab2=f32(inputs["a_b2"]).reshape(-1, 1),
        aw3T=f32(inputs["a_w3"].T), ab3=f32(inputs["a_b3"]).reshape(-1, 1),
        vw1T=f32(inputs["v_w1"].T), vb1=f32(inputs["v_b1"]).reshape(-1, 1),
        vw2T=f32(inputs["v_w2"].T), vb2=f32(inputs["v_b2"]).reshape(-1, 1),
        vw3T=f32(inputs["v_w3"].T), vb3=f32(inputs["v_b3"]).reshape(-1, 1),
        w2T=bf16(f32(inputs["pf_w2"]).T),
        b1=f32(inputs["pf_b1"]).reshape(1, -1), b2=f32(inputs["pf_b2"]).reshape(1, -1),
        w3=f32(inputs["pf_w3"]).reshape(1, -1), b3=f32(inputs["pf_b3"]).reshape(1, 1),
        id128=np.eye(128, dtype=np.float32),
    )
    sel = np.zeros((AH + 1, NCH * 128), np.float32)
    for r in range(SLAB):
        sel[r // (VH + 1), r] = 1.0
    common["selT"] = sel
    twTall = np.zeros((TH, NCORES * TPC), np.float32)
    tblall = np.zeros((NCORES * TPC, 1), np.float32)
    tblall[0, 0] = 1.0
    twTall[:, 1:TOUT + 1] = t_w.T
    tblall[1:TOUT + 1, 0] = t_b
    common["twTa"] = bf16(twTall[:, 0:TH])
    common["twTb"] = bf16(twTall[:, TH:TH + NCORES])
    common["tbla"] = f32(tblall[0:TH])
    common["tblb"] = f32(tblall[TH:TH + NCORES])

    in_maps = []
    for i in range(NCORES):
        m = dict(common)
        sh = text_x[i * BL:(i + 1) * BL]                      # [16, 50, 300]
        m["textxt"] = bf16(sh.transpose(2, 1, 0).reshape(DT, S * BL))
        # per-core W1 slab: global t in [i*TPC, (i+1)*TPC), zero-padded past t=128
        w1t = np.zeros((KLOC, PF), np.float32)
        for tau in range(TPC):
            tg = i * TPC + tau
            if tg < (TOUT + 1):
                w1t[tau * SLAB:(tau + 1) * SLAB] = pf_w1[:, tg * SLAB:(tg + 1) * SLAB].T
        m["w1T"] = bf16(w1t)
        in_maps.append(m)
    return in_maps


_CACHE = {}


def kernel(**inputs):
    if "nc" not in _CACHE:
        _CACHE["nc"] = build_nc()
    nc = _CACHE["nc"]
    in_maps = make_in_maps(inputs)
    res = run_bass_kernel_spmd(nc, in_maps, core_ids=list(range(NCORES)))
    return np.asarray(res.results[0]["out"], dtype=np.float32)


if __name__ == "__main__":
    import reference
    inputs = {k: np.asarray(v) for k, v in reference.setup_inputs().items()}
    out = kernel(**inputs)
    exp = np.asarray(reference.reference(**inputs))
    err = np.linalg.norm(out - exp) / np.linalg.norm(exp)
    print("Relative error:", err)



# revision 22
# speedup vs baseline: 1.6241x; 1.0992x over previous
import numpy as np
import ml_dtypes
from contextlib import ExitStack

import concourse.bass as bass
import concourse.tile as tile
from concourse import bacc, mybir
from concourse.bass_utils import run_bass_kernel_spmd

F32 = mybir.dt.float32
BF16 = mybir.dt.bfloat16

B, S, DT = 128, 50, 300
AIN, VIN = 74, 35
AH, VH, TH = 32, 32, 128
TOUT = 128
PF = 768
NCORES = 8
BL = B // NCORES          # 16 batch per core (LSTM shard)
TPC = 17                  # t-values per core (8*17=136 >= 129, zero-padded)
SLAB = (AH + 1) * (VH + 1)  # 1089 = fusion rows per t
NCH = 9                   # ceil(1089/128) chunks per slab; last chunk 65 rows
LASTK = SLAB - 8 * 128    # 65
KLOC = TPC * SLAB         # 18513 k rows per core
GPERM = [0, 1, 3, 2]      # torch gate order i,f,g,o -> i,f,o,g

import os
BUILD_PHASE = int(os.environ.get("BUILD_PHASE", "8"))


def _phases():
    # partition phase of vT needed for each chunk c: v = (p + 128c) mod 33
    return [(128 * c) % 33 for c in range(NCH)]


def build_nc():
    nc = bacc.Bacc(None, target_bir_lowering=False)

    # ---- DRAM parameters (identical shapes on all cores) ----
    P = {}
    def par(name, shape, dt=F32):
        P[name] = nc.declare_dram_parameter(name, list(shape), dt, isOutput=False)
        return P[name]

    par("textxt", (DT, S * BL), BF16)    # [300, 800] (d, (s,b))
    par("wihT", (DT, 4 * TH), BF16)      # [300, 512] gate-permuted
    par("biaspp", (4, TH, 1))            # [4,128,1]
    par("whhT", (TH, 4 * TH), BF16)      # [128, 512] gate-permuted
    par("twTa", (TH, TH), BF16)          # t_w cols for global t 0..127
    par("twTb", (TH, NCORES), BF16)      # global t 128..135 (pads zero)
    par("tbla", (TH, 1))                 # bias col (1.0 at t==0)
    par("tblb", (NCORES, 1))
    par("audio", (B, AIN))
    par("video", (B, VIN))
    par("agam", (1, AIN)); par("abet", (1, AIN))
    par("vgam", (1, VIN)); par("vbet", (1, VIN))
    par("aw1T", (AIN, AH)); par("ab1", (AH, 1))
    par("aw2T", (AH, AH)); par("ab2", (AH, 1))
    par("aw3T", (AH, AH)); par("ab3", (AH, 1))
    par("vw1T", (VIN, VH)); par("vb1", (VH, 1))
    par("vw2T", (VH, VH)); par("vb2", (VH, 1))
    par("vw3T", (VH, VH)); par("vb3", (VH, 1))
    par("w1T", (KLOC, PF), BF16)         # [18513, 768] per-core pre-transposed slab
    par("selT", (AH + 1, NCH * 128))     # a-index selector: selT[a, c*128+p]=1 iff (128c+p)//33==a
    par("w2T", (PF, PF), BF16)
    par("b1", (1, PF)); par("b2", (1, PF))
    par("w3", (1, PF)); par("b3", (1, 1))
    par("id128", (128, 128))
    par("id128b", (128, 128), BF16)
    out_ext = nc.declare_dram_parameter("out", [B, 1], F32, isOutput=True)
    DBG = os.environ.get("KERNEL_DEBUG", "0") == "1"
    dbg = {}
    if DBG:
        for n, sh in [("dbg_h", (TH, BL)), ("dbg_th", (NCORES * TPC, BL)),
                      ("dbg_tT", (1, TPC * B)), ("dbg_av", (128, NCH * B)),
                      ("dbg_y1", (B, PF)), ("dbg_y1g", (B, PF)),
                      ("dbg_xp", (TH, 4 * S * BL))]:
            dbg[n] = nc.declare_dram_parameter(n, list(sh), F32, isOutput=True)

    with tile.TileContext(nc) as tc, ExitStack() as ctx:
        ep = ctx.enter_context
        const = ep(tc.tile_pool(name="const", bufs=1))
        w1p = ep(tc.tile_pool(name="w1p", bufs=80))
        work = ep(tc.tile_pool(name="work", bufs=2))
        dram = ep(tc.tile_pool(name="dram", bufs=1, space="DRAM"))

        def load(name, shape, dt=F32, pool=const):
            t = pool.tile(list(shape), dt, tag=name)
            nc.gpsimd.dma_start(t[:], P[name][:])
            return t

        ones1 = const.tile([1, 128], F32, tag="ones1")
        nc.vector.memset(ones1[:], 1.0)
        ones1b = const.tile([1, 128], BF16, tag="ones1b")
        nc.vector.memset(ones1b[:], 1.0)

        def pbcast(psum_pool, row_ap, n, tag="bc"):
            # replicate [1, n] row to [128, n] via K=1 outer product on PE
            bc = psum_pool.tile([128, n], F32, tag=tag)
            nc.tensor.matmul(bc[:], ones1[:], row_ap, start=True, stop=True)
            return bc

        # ---- small input loads ----
        idm = load("id128", (128, 128))
        idmb = load("id128b", (128, 128), BF16)
        txc, wihc = [], []
        kcs = [128, 128, DT - 256]
        for ci, kc in enumerate(kcs):
            t = const.tile([kc, S * BL], BF16, tag=f"txc{ci}")
            nc.gpsimd.dma_start(t[:], P["textxt"][ci * 128:ci * 128 + kc, :])
            txc.append(t)
            w = const.tile([kc, 4 * TH], BF16, tag=f"wihc{ci}")
            nc.gpsimd.dma_start(w[:], P["wihT"][ci * 128:ci * 128 + kc, :])
            wihc.append(w)
        whh = load("whhT", (TH, 4 * TH), BF16)
        biasj = []
        for j in range(4):
            bt = const.tile([TH, 1], F32, tag=f"biasj{j}")
            nc.gpsimd.dma_start(bt[:], P["biaspp"][j, :, :])
            biasj.append(bt)
        twTa = load("twTa", (TH, TH), BF16); twTb = load("twTb", (TH, NCORES), BF16)
        tbla = load("tbla", (TH, 1)); tblb = load("tblb", (NCORES, 1))
        aud = load("audio", (B, AIN)); vid = load("video", (B, VIN))
        agam = load("agam", (1, AIN)); abet = load("abet", (1, AIN))
        vgam = load("vgam", (1, VIN)); vbet = load("vbet", (1, VIN))
        subw = {n: load(n, sh) for n, sh in [
            ("aw1T", (AIN, AH)), ("ab1", (AH, 1)), ("aw2T", (AH, AH)), ("ab2", (AH, 1)),
            ("aw3T", (AH, AH)), ("ab3", (AH, 1)),
            ("vw1T", (VIN, VH)), ("vb1", (VH, 1)), ("vw2T", (VH, VH)), ("vb2", (VH, 1)),
            ("vw3T", (VH, VH)), ("vb3", (VH, 1))]}
        w2c = []
        for j in range(6):
            t = const.tile([128, PF], BF16, tag=f"w2c{j}")
            nc.gpsimd.dma_start(t[:], P["w2T"][j * 128:(j + 1) * 128, :])
            w2c.append(t)
        b1r = load("b1", (1, PF)); b2r = load("b2", (1, PF))
        w3r = load("w3", (1, PF)); b3r = load("b3", (1, 1))

        # ---- W1 streaming DMAs (emitted early; pool slots throttle) ----
        w1tiles = []
        for tau in range(TPC):
            for c in range(NCH):
                kk = 128 if c < 8 else LASTK
                r0 = tau * SLAB + c * 128
                t = w1p.tile([128, PF], BF16, tag="w1chunk")
                nc.sync.dma_start(t[0:kk, :], P["w1T"][r0:r0 + kk, :])
                w1tiles.append(t)

        # ---- audio/video subnets (redundant, full batch), transposed outputs ----
        def subnet(x, gam, bet, nin, nh, w1, b1, w2, b2, w3, b3, out3):
            with tc.tile_pool(name="sn", bufs=2) as sn, \
                 tc.tile_pool(name="snp", bufs=2, space="PSUM") as snp:
                gb = pbcast(snp, gam[0:1, :], nin, tag="gbc")
                bb = pbcast(snp, bet[0:1, :], nin, tag="gbc")
                aff = sn.tile([B, nin], F32, tag="aff")
                nc.vector.tensor_mul(aff[:], x[:], gb[:])
                nc.vector.tensor_add(aff[:], aff[:], bb[:])
                pT = snp.tile([nin, B], F32, tag="pT")
                nc.tensor.transpose(pT[:], aff[:], idm[:])
                xT = sn.tile([nin, B], F32, tag="xT")
                nc.vector.tensor_copy(xT[:], pT[:])
                h = xT
                for wi, bi in ((w1, b1), (w2, b2), (w3, b3)):
                    hp = snp.tile([nh, B], F32, tag="hp")
                    nc.tensor.matmul(hp[:], wi[:], h[:], start=True, stop=True)
                    dst = out3 if wi is w3 else sn.tile([nh, B], F32, tag="hs")
                    nc.scalar.activation(dst[:], hp[:], mybir.ActivationFunctionType.Relu,
                                         bias=bi[:])
                    h = dst

        ah3 = const.tile([AH, B], F32, tag="ah3")
        subnet(aud, agam, abet, AIN, AH, subw["aw1T"], subw["ab1"], subw["aw2T"],
               subw["ab2"], subw["aw3T"], subw["ab3"], ah3)
        vh3 = const.tile([VH, B], F32, tag="vh3")
        subnet(vid, vgam, vbet, VIN, VH, subw["vw1T"], subw["vb1"], subw["vw2T"],
               subw["vb2"], subw["vw3T"], subw["vb3"], vh3)
        # vT/aT = [ones; h^T] (partition layout, via DMA partition shift)
        vT = const.tile([VH + 1, B], F32, tag="vT")
        nc.vector.memset(vT[0:1, :], 1.0)
        nc.gpsimd.dma_start(vT[1:VH + 1, :], vh3[:, :])
        aT33 = const.tile([AH + 1, B], F32, tag="aT33")
        nc.vector.memset(aT33[0:1, :], 1.0)
        nc.gpsimd.dma_start(aT33[1:AH + 1, :], ah3[:, :])
        selT = load("selT", (AH + 1, NCH * 128))

        # ---- x_proj: xpT_all[g%128, j, (s,b)] = (w_ih x + b)^T, gate-permuted ----
        xpT = const.tile([TH, 4, S * BL], BF16, tag="xpT")
        with tc.tile_pool(name="xpp", bufs=2, space="PSUM") as xpp:
            for j in range(4):
                ps0 = xpp.tile([TH, 400], F32, tag="xps0")
                ps1 = xpp.tile([TH, 400], F32, tag="xps1")
                ps = [ps0, ps1]
                for ci in range(3):
                    for half in range(2):
                        nc.tensor.matmul(
                            ps[half][:],
                            wihc[ci][:, j * TH:(j + 1) * TH],
                            txc[ci][:, half * 400:(half + 1) * 400],
                            start=(ci == 0), stop=(ci == 2))
                for half in range(2):
                    nc.scalar.activation(xpT[:, j, half * 400:(half + 1) * 400],
                                         ps[half][:],
                                         mybir.ActivationFunctionType.Identity,
                                         bias=biasj[j][:])

        # ---- LSTM recurrence (transposed state, 16 batch cols) ----
        hT = const.tile([TH, BL], BF16, tag="hT")
        cT = const.tile([TH, BL], F32, tag="cT")
        nc.vector.memset(hT[:], 0.0)
        nc.vector.memset(cT[:], 0.0)
        with tc.tile_pool(name="gp", bufs=2, space="PSUM") as gp, \
             tc.tile_pool(name="gs", bufs=2) as gs:
            for t in range(S):
                pg = gp.tile([TH, 4, BL], F32, tag="pg")
                # x-projection added via PE accumulate (identity matmul) so the
                # per-step vector add leaves the serial chain
                nc.tensor.matmul(pg[:], idmb[:], xpT[:, :, t * BL:(t + 1) * BL],
                                 start=True, stop=False, skip_group_check=True)
                for j in range(4):
                    nc.tensor.matmul(pg[:, j, :], whh[:, j * TH:(j + 1) * TH], hT[:],
                                     start=False, stop=(j == 3), skip_group_check=True)
                ga = gs.tile([TH, 4, BL], F32, tag="ga")
                nc.scalar.activation(ga[:, 0:3, :], pg[:, 0:3, :],
                                     mybir.ActivationFunctionType.Sigmoid)
                nc.scalar.activation(ga[:, 3, :], pg[:, 3, :],
                                     mybir.ActivationFunctionType.Tanh)
                t1 = gs.tile([TH, BL], F32, tag="t1")
                nc.vector.tensor_mul(t1[:], ga[:, 1, :], cT[:])      # sig(f)*c
                t2 = gs.tile([TH, BL], F32, tag="t2")
                nc.vector.tensor_mul(t2[:], ga[:, 0, :], ga[:, 3, :])  # sig(i)*tanh(g)
                nc.vector.tensor_add(cT[:], t1[:], t2[:])
                tc2 = gs.tile([TH, BL], F32, tag="tc2")
                nc.scalar.activation(tc2[:], cT[:], mybir.ActivationFunctionType.Tanh)
                nc.vector.tensor_mul(hT[:], ga[:, 2, :], tc2[:])     # sig(o)*tanh(c)

        # ---- all 136 text rows over local batch shard + AllToAll ----
        # After AllToAll, core i holds its own 17 t-rows over the FULL batch.
        tT = const.tile([1, TPC * B], BF16, tag="tT")
        with tc.tile_pool(name="thp", bufs=2, space="PSUM") as thp:
            ph1 = thp.tile([TH, BL], F32, tag="ph1")
            nc.tensor.matmul(ph1[:], twTa[:], hT[:], start=True, stop=True)
            ths1 = work.tile([TH, BL], BF16, tag="ths1")
            nc.scalar.activation(ths1[:], ph1[:], mybir.ActivationFunctionType.Identity,
                                 bias=tbla[:])
            ph2 = thp.tile([NCORES, BL], F32, tag="ph2")
            nc.tensor.matmul(ph2[:], twTb[:], hT[:], start=True, stop=True)
            ths2 = work.tile([NCORES, BL], BF16, tag="ths2")
            nc.scalar.activation(ths2[:], ph2[:], mybir.ActivationFunctionType.Identity,
                                 bias=tblb[:])
        if DBG:
            nc.sync.dma_start(dbg["dbg_h"][:], hT[:])
            nc.sync.dma_start(dbg["dbg_xp"][:], xpT[:])
        agin = dram.tile([NCORES * TPC, BL], BF16, tag="agin")
        agout = dram.tile([NCORES, TPC, BL], BF16, tag="agout")
        nc.gpsimd.dma_start(agin[0:TH, :], ths1[:])
        nc.gpsimd.dma_start(agin[TH:TH + NCORES, :], ths2[:])
        nc.gpsimd.collective_compute(
            "AllToAll", mybir.AluOpType.bypass,
            replica_groups=[list(range(NCORES))],
            ins=[agin[:].opt()], outs=[agout[:].opt()])
        for tau in range(TPC):
            nc.gpsimd.dma_start(tT[0:1, tau * B:(tau + 1) * B], agout[:, tau, :])

        if DBG:
            nc.sync.dma_start(dbg["dbg_th"][0:TH, :], ths1[:])
            nc.sync.dma_start(dbg["dbg_th"][TH:TH + NCORES, :], ths2[:])
            nc.sync.dma_start(dbg["dbg_tT"][:], tT[:])
        # ---- avT [128, 9, 128]: av outer product, fusion-chunk layout ----
        # vT replicated/phase-shifted tiles (DMA partition shifts)
        phases = sorted(set(_phases()))
        vrep = {}
        for ph_ in phases:
            vr = const.tile([128, B], F32, tag=f"vrep{ph_}")
            p = 0
            v = ph_
            while p < 128:
                ln = min(33 - v, 128 - p)
                nc.gpsimd.dma_start(vr[p:p + ln, :], vT[v:v + ln, :])
                p += ln
                v = (v + ln) % 33
            vrep[ph_] = vr
        avT = const.tile([128, NCH, B], BF16, tag="avT")
        with tc.tile_pool(name="bca", bufs=3, space="PSUM") as bca:
            for c in range(NCH):
                ph_ = (128 * c) % 33
                vr = vrep[ph_]
                arep = bca.tile([128, B], F32, tag="arep")
                nc.tensor.matmul(arep[:], selT[:, c * 128:(c + 1) * 128], aT33[:],
                                 start=True, stop=True)
                nc.vector.tensor_mul(avT[:, c, :], vr[:], arep[:])

        # ---- fusion slabs (bf16) + big matmul, accumulate y1 in PSUM ----
        y1sb = const.tile([B, PF], F32, tag="y1sb")
        with tc.tile_pool(name="y1pp", bufs=1, space="PSUM") as y1pp, \
             tc.tile_pool(name="fus", bufs=3) as fusp, \
             tc.tile_pool(name="tbc", bufs=3, space="PSUM") as tbcp:
            psY0 = y1pp.tile([B, 384], F32, tag="psY0")
            psY1 = y1pp.tile([B, 384], F32, tag="psY1")
            psYh = [psY0, psY1]
            wi = 0
            for tau in range(TPC):
                tb = tbcp.tile([128, B], F32, tag="tb")
                nc.tensor.matmul(tb[:], ones1b[:], tT[0:1, tau * B:(tau + 1) * B],
                                 start=True, stop=True)
                tbs = fusp.tile([128, B], BF16, tag="tbs")
                nc.vector.tensor_copy(tbs[:], tb[:])
                fu = fusp.tile([128, NCH, B], BF16, tag="fu")
                for c in range(NCH):
                    nc.vector.tensor_mul(fu[:, c, :], avT[:, c, :], tbs[:])
                for c in range(NCH):
                    kk = 128 if c < 8 else LASTK
                    w1t = w1tiles[wi]; wi += 1
                    first = (tau == 0 and c == 0)
                    last = (tau == TPC - 1 and c == NCH - 1)
                    nc.tensor.matmul(psYh[0][:], fu[0:kk, c, :], w1t[0:kk, 0:384],
                                     start=first, stop=last)
                    nc.tensor.matmul(psYh[1][:], fu[0:kk, c, :], w1t[0:kk, 384:768],
                                     start=first, stop=last)

            nc.vector.tensor_copy(y1sb[:, 0:384], psYh[0][:])
            nc.vector.tensor_copy(y1sb[:, 384:768], psYh[1][:])
        if DBG:
            nc.sync.dma_start(dbg["dbg_av"][:], avT[:])
            nc.sync.dma_start(dbg["dbg_y1"][:], y1sb[:])

        # ---- AllReduce y1 ----
        arin = dram.tile([B, PF], F32, tag="arin")
        arout = dram.tile([B, PF], F32, tag="arout")
        nc.gpsimd.dma_start(arin[:], y1sb[:])
        nc.gpsimd.collective_compute(
            "AllReduce", mybir.AluOpType.add,
            replica_groups=[list(range(NCORES))],
            ins=[arin[:].opt()], outs=[arout[:].opt()])
        y1g = const.tile([B, PF], F32, tag="y1g")
        nc.gpsimd.dma_start(y1g[:], arout[:])
        if DBG:
            nc.sync.dma_start(dbg["dbg_y1g"][:], y1g[:])

        # ---- epilogue: bias+relu, transpose, layer2, layer3, sigmoid ----
        y1r = const.tile([B, PF], F32, tag="y1r")
        with tc.tile_pool(name="ep1", bufs=2, space="PSUM") as ep1:
            for h in range(2):
                bh = ep1.tile([B, 384], F32, tag="epb")
                nc.tensor.matmul(bh[:], ones1[:], b1r[0:1, h * 384:(h + 1) * 384],
                                 start=True, stop=True)
                nc.vector.tensor_add(y1g[:, h * 384:(h + 1) * 384],
                                     y1g[:, h * 384:(h + 1) * 384], bh[:])
            nc.vector.tensor_scalar_max(y1r[:], y1g[:], 0.0)

        y1T = const.tile([128, 6, B], BF16, tag="y1T")
        with tc.tile_pool(name="trp", bufs=2, space="PSUM") as trp:
            for j in range(6):
                pt = trp.tile([128, B], F32, tag="pt")
                nc.tensor.transpose(pt[:], y1r[:, j * 128:(j + 1) * 128], idm[:])
                nc.vector.tensor_copy(y1T[:, j, :], pt[:])

        with tc.tile_pool(name="y2pp", bufs=1, space="PSUM") as y2pp:
            ps20 = y2pp.tile([B, 384], F32, tag="ps20")
            ps21 = y2pp.tile([B, 384], F32, tag="ps21")
            ps2h = [ps20, ps21]
            y2 = const.tile([B, PF], F32, tag="y2")
            for h in range(2):
                for j in range(6):
                    nc.tensor.matmul(ps2h[h][:], y1T[:, j, :],
                                     w2c[j][:, h * 384:(h + 1) * 384],
                                     start=(j == 0), stop=False)
                # bias via accumulating ones x b2 outer product, then relu from PSUM
                nc.tensor.matmul(ps2h[h][:], ones1[:], b2r[0:1, h * 384:(h + 1) * 384],
                                 start=False, stop=True)
                nc.vector.tensor_scalar_max(y2[:, h * 384:(h + 1) * 384],
                                            ps2h[h][:], 0.0)

        zb = const.tile([B, 1], F32, tag="zb")
        with tc.tile_pool(name="ep3", bufs=2, space="PSUM") as ep3:
            prod = const.tile([B, PF], F32, tag="prod")
            for h in range(2):
                wh = ep3.tile([B, 384], F32, tag="epb")
                nc.tensor.matmul(wh[:], ones1[:], w3r[0:1, h * 384:(h + 1) * 384],
                                 start=True, stop=True)
                nc.vector.tensor_mul(prod[:, h * 384:(h + 1) * 384],
                                     y2[:, h * 384:(h + 1) * 384], wh[:])
            scr = const.tile([B, PF], F32, tag="scr")
            zacc = const.tile([B, 1], F32, tag="zacc")
            nc.scalar.activation(scr[:], prod[:], mybir.ActivationFunctionType.Identity,
                                 accum_out=zacc[:])
            b3b = ep3.tile([B, 1], F32, tag="b3s")
            nc.tensor.matmul(b3b[:], ones1[:], b3r[0:1, :], start=True, stop=True)
            nc.vector.tensor_add(zb[:], zacc[:], b3b[:])
        zs = const.tile([B, 1], F32, tag="zs")
        nc.scalar.activation(zs[:], zb[:], mybir.ActivationFunctionType.Sigmoid)

        zf = const.tile([B, 1], F32, tag="zf")
        nc.vector.tensor_scalar(zf[:], zs[:], 6.0, -3.0,
                                mybir.AluOpType.mult, mybir.AluOpType.add)
        nc.sync.dma_start(out_ext[:], zf[:])

    nc.compile()
    return nc


def make_in_maps(inputs):
    f32 = lambda a: np.ascontiguousarray(a, dtype=np.float32)
    bf16 = lambda a: np.ascontiguousarray(a.astype(ml_dtypes.bfloat16))
    perm = np.concatenate([np.arange(g * TH, (g + 1) * TH) for g in GPERM])

    text_x = f32(inputs["text_x"])
    w_ihp = f32(inputs["w_ih"])[perm]          # [512, 300]
    w_hhp = f32(inputs["w_hh"])[perm]          # [512, 128]
    biaspp = (f32(inputs["b_ih"]) + f32(inputs["b_hh"]))[perm].reshape(4, TH, 1)
    t_w = f32(inputs["t_w"]); t_b = f32(inputs["t_b"])
    pf_w1 = np.asarray(inputs["pf_w1"], dtype=np.float32)

    common = dict(
        wihT=bf16(w_ihp.T), biaspp=f32(biaspp), whhT=bf16(w_hhp.T),
        audio=f32(inputs["audio_x"][:, 0, :]), video=f32(inputs["video_x"][:, 0, :]),
        agam=f32(inputs["a_gamma"]).reshape(1, -1), abet=f32(inputs["a_beta"]).reshape(1, -1),
        vgam=f32(inputs["v_gamma"]).reshape(1, -1), vbet=f32(inputs["v_beta"]).reshape(1, -1),
        aw1T=f32(inputs["a_w1"].T), ab1=f32(inputs["a_b1"]).reshape(-1, 1),
        aw2T=f32(inputs["a_w2"].T), ab2=f32(inputs["a_b2"]).reshape(-1, 1),
        aw3T=f32(inputs["a_w3"].T), ab3=f32(inputs["a_b3"]).reshape(-1, 1),
        vw1T=f32(inputs["v_w1"].T), vb1=f32(inputs["v_b1"]).reshape(-1, 1),
        vw2T=f32(inputs["v_w2"].T), vb2=f32(inputs["v_b2"]).reshape(-1, 1),
        vw3T=f32(inputs["v_w3"].T), vb3=f32(inputs["v_b3"]).reshape(-1, 1),
        w2T=bf16(f32(inputs["pf_w2"]).T),
        b1=f32(inputs["pf_b1"]).reshape(1, -1), b2=f32(inputs["pf_b2"]).reshape(1, -1),
        w3=f32(inputs["pf_w3"]).reshape(1, -1), b3=f32(inputs["pf_b3"]).reshape(1, 1),
        id128=np.eye(128, dtype=np.float32),
        id128b=np.eye(128, dtype=ml_dtypes.bfloat16),
    )
    sel = np.zeros((AH + 1, NCH * 128), np.float32)
    for r in range(SLAB):
        sel[r // (VH + 1), r] = 1.0
    common["selT"] = sel
    twTall = np.zeros((TH, NCORES * TPC), np.float32)
    tblall = np.zeros((NCORES * TPC, 1), np.float32)
    tblall[0, 0] = 1.0
    twTall[:, 1:TOUT + 1] = t_w.T
    tblall[1:TOUT + 1, 0] = t_b
    common["twTa"] = bf16(twTall[:, 0:TH])
    common["twTb"] = bf16(twTall[:, TH:TH + NCORES])
    common["tbla"] = f32(tblall[0:TH])
    common["tblb"] = f32(tblall[TH:TH + NCORES])

    in_maps = []
    for i in range(NCORES):
        m = dict(common)
        sh = text_x[i * BL:(i + 1) * BL]                      # [16, 50, 300]
        m["textxt"] = bf16(sh.transpose(2, 1, 0).reshape(DT, S * BL))
        # per-core W1 slab: global t in [i*TPC, (i+1)*TPC), zero-padded past t=128
        w1t = np.zeros((KLOC, PF), np.float32)
        for tau in range(TPC):
            tg = i * TPC + tau
            if tg < (TOUT + 1):
                w1t[tau * SLAB:(tau + 1) * SLAB] = pf_w1[:, tg * SLAB:(tg + 1) * SLAB].T
        m["w1T"] = bf16(w1t)
        in_maps.append(m)
    return in_maps


_CACHE = {}


def kernel(**inputs):
    if "nc" not in _CACHE:
        _CACHE["nc"] = build_nc()
    nc = _CACHE["nc"]
    in_maps = make_in_maps(inputs)
    res = run_bass_kernel_spmd(nc, in_maps, core_ids=list(range(NCORES)))
    return np.asarray(res.results[0]["out"], dtype=np.float32)


if __name__ == "__main__":
    import reference
    inputs = {k: np.asarray(v) for k, v in reference.setup_inputs().items()}
    out = kernel(**inputs)
    exp = np.asarray(reference.reference(**inputs))
    err = np.linalg.norm(out - exp) / np.linalg.norm(exp)
    print("Relative error:", err)

